# revision 6
# baseline (speedup 1.0000x reference)
"""Bass/Trainium2 kernel for nn_Attention (8 heads, N=M=4096, SENT=1024, HD=128).

Strategy: head-sharded over 8 NeuronCores (core h owns head h).
Per core:
  P : S^T = (W_s^T @ sentences^T) + b_s   [128,4096] f32r   (e on partitions)
      K^T = (W_k^T @ knowledge^T) + b_k   [128,4096] f32r
  A : scoresT tiles [m,n] = K^T(slice).T @ S^T, exp(scale*x) -> E bf16 -> DRAM,
      rowsum over free axis (softmax over sentences n = free axis) via DVE on
      the bf16 E tiles, recip, K' = knowledge * recip[m]
  B1+B2 interleaved:
      B1: attn_outT[d,n] = sum_m K'[m,d] * E[m,n]  (4 PSUM banks, 2 d-halves)
      B2: weights_cat recomputed in [n,m] orientation:
          exp(S^T(slice).T @ K^T) * recip_broadcast -> fp32 -> DRAM
  C : F = attn_outT.T @ W_f_h + b_f/8 (bf16) ; column-chunked ReduceScatter
      overlapped with C; fp32 upcast of the RS result on-chip.
Host: concatenates per-core weights blocks (head-major) and RS row chunks.
"""

import numpy as np

H, SENT, HD = 8, 1024, 128
N, M = 4096, 4096
NCORES = 8
SCALE = 1.0 / float(np.sqrt(HD))

_CACHE = {}


def _build():
    from contextlib import ExitStack
    import concourse.bass as bass
    import concourse.mybir as mybir
    import concourse.tile as tile
    from concourse import bacc

    F32 = mybir.dt.float32
    F32R = mybir.dt.float32r
    BF16 = mybir.dt.bfloat16
    AX = mybir.AxisListType.X
    EXP = mybir.ActivationFunctionType.Exp
    COPY = mybir.ActivationFunctionType.Copy

    nc = bacc.Bacc("TRN2", target_bir_lowering=False, debug=False, num_devices=NCORES)

    sentT = nc.dram_tensor("sentT", [SENT, N], F32, kind="ExternalInput")
    knowT = nc.dram_tensor("knowT", [SENT, M], F32, kind="ExternalInput")
    know = nc.dram_tensor("know", [M, SENT], F32, kind="ExternalInput")
    Ws = nc.dram_tensor("Ws", [SENT, HD], F32, kind="ExternalInput")
    bs = nc.dram_tensor("bs", [HD, 1], F32, kind="ExternalInput")
    Wk = nc.dram_tensor("Wk", [SENT, HD], F32, kind="ExternalInput")
    bk = nc.dram_tensor("bk", [HD, 1], F32, kind="ExternalInput")
    Wf = nc.dram_tensor("Wf", [SENT, SENT], F32, kind="ExternalInput")
    bf = nc.dram_tensor("bf", [1, SENT], F32, kind="ExternalInput")

    w_out = nc.dram_tensor("weights_out", [N, M], F32, kind="ExternalOutput")
    out_part = nc.dram_tensor("out_part", [N // NCORES, SENT], F32, kind="ExternalOutput")

    MT = M // 128   # 32 m tiles
    NT = N // 128   # 32 n tiles
    NCH = N // 512  # 8 n chunks
    MCH = M // 512  # 8 m chunks
    DT = SENT // 128  # 8 d tiles
    NPART = N // NCORES  # 512 rows per core after RS
    CC = 2               # column chunks for overlapped ReduceScatter

    with tile.TileContext(nc) as tc:
        with ExitStack() as glob_ctx:
            glob = glob_ctx.enter_context(tc.tile_pool(name="glob", bufs=1))
            dram = glob_ctx.enter_context(tc.tile_pool(name="dram", bufs=1, space="DRAM"))

            E_dram = dram.tile([M, N], BF16)
            recip_dram = dram.tile([1, M], F32)
            F_chunks = [
                dram.tile([N, SENT // CC], BF16, name=f"Fch{c}") for c in range(CC)
            ]
            rs_chunks = [
                dram.tile([NPART, SENT // CC], BF16, name=f"rsch{c}") for c in range(CC)
            ]

            recip_all = glob.tile([128, MT], F32)
            kp = glob.tile([128, MT * SENT], BF16)   # K' blocks: [:, mt*SENT + d]
            attn_sb = glob.tile([128, DT * N], BF16)  # [:, d*N + n]

            with ExitStack() as stp:
                projc = stp.enter_context(tc.tile_pool(name="projc", bufs=1))
                S_sb = projc.tile([128, N], F32R)
                K_sb = projc.tile([128, M], F32R)

                # ============ Phase P: projections (j-outer, 8 psum banks) ========
                with ExitStack() as stp2:
                    wpool = stp2.enter_context(tc.tile_pool(name="wpool", bufs=1))
                    ppool = stp2.enter_context(tc.tile_pool(name="ppool", bufs=2))
                    psP = stp2.enter_context(tc.tile_pool(name="psP", bufs=1, space="PSUM"))

                    Ws_sb = wpool.tile([128, DT * HD], F32R)
                    Wk_sb = wpool.tile([128, DT * HD], F32R)
                    bs_sb = wpool.tile([128, 1], F32)
                    bk_sb = wpool.tile([128, 1], F32)
                    nc.sync.dma_start(out=bs_sb[:], in_=bs[:])
                    nc.sync.dma_start(out=bk_sb[:], in_=bk[:])
                    for j in range(DT):
                        wr = ppool.tile([128, HD], F32, tag="wraw")
                        nc.sync.dma_start(out=wr[:], in_=Ws[j * 128:(j + 1) * 128, :])
                        nc.vector.tensor_copy(Ws_sb[:, j * HD:(j + 1) * HD], wr[:])
                        wr2 = ppool.tile([128, HD], F32, tag="wraw")
                        nc.sync.dma_start(out=wr2[:], in_=Wk[j * 128:(j + 1) * 128, :])
                        nc.vector.tensor_copy(Wk_sb[:, j * HD:(j + 1) * HD], wr2[:])

                    for dst, src_dram, wsb, bcol, ldeng in (
                        (S_sb, sentT, Ws_sb, bs_sb, nc.gpsimd),
                        (K_sb, knowT, Wk_sb, bk_sb, nc.sync),
                    ):
                        pss = [
                            psP.tile([128, 512], F32, tag=f"pp{q}", name=f"pp{q}")
                            for q in range(NCH)
                        ]
                        for j in range(DT):
                            for hh in range(4):  # 1024-col slabs
                                raw = ppool.tile([128, 1024], F32, tag="praw")
                                ldeng.dma_start(
                                    out=raw[:],
                                    in_=src_dram[j * 128:(j + 1) * 128,
                                                 hh * 1024:(hh + 1) * 1024],
                                )
                                cvt = ppool.tile([128, 1024], F32R, tag="pcvt")
                                nc.vector.tensor_copy(cvt[:], raw[:])
                                for qq in range(2):
                                    q = hh * 2 + qq
                                    nc.tensor.matmul(
                                        pss[q][:], wsb[:, j * HD:(j + 1) * HD],
                                        cvt[:, qq * 512:(qq + 1) * 512],
                                        start=(j == 0), stop=(j == DT - 1),
                                    )
                        for q in range(NCH):
                            nc.vector.tensor_scalar_add(
                                dst[:, q * 512:(q + 1) * 512], pss[q][:], bcol[:]
                            )

                # ============ Phase A: scores/exp/rowsum + K' prep ================
                with ExitStack() as sta:
                    apool = sta.enter_context(tc.tile_pool(name="apool", bufs=2))
                    kpool = sta.enter_context(tc.tile_pool(name="kpool", bufs=3))
                    spool = sta.enter_context(tc.tile_pool(name="spool", bufs=4))
                    psA = sta.enter_context(tc.tile_pool(name="psA", bufs=4, space="PSUM"))

                    for mt in range(MT):
                        part = spool.tile([128, NCH], F32, tag="part")
                        erow = apool.tile([128, N], BF16, tag="erow")
                        for nch in range(NCH):
                            ps = psA.tile([128, 512], F32, tag="ps")
                            nc.tensor.matmul(
                                ps[:], K_sb[:, mt * 128:(mt + 1) * 128],
                                S_sb[:, nch * 512:(nch + 1) * 512],
                                start=True, stop=True,
                            )
                            nc.scalar.activation(
                                erow[:, nch * 512:(nch + 1) * 512], ps[:], EXP,
                                scale=SCALE,
                            )
                            nc.vector.reduce_sum(
                                part[:, nch:nch + 1],
                                erow[:, nch * 512:(nch + 1) * 512], axis=AX,
                            )
                        nc.sync.dma_start(
                            out=E_dram[mt * 128:(mt + 1) * 128, :], in_=erow[:]
                        )
                        rsum = spool.tile([128, 1], F32, tag="rsum")
                        nc.vector.reduce_sum(rsum[:], part[:], axis=AX)
                        nc.vector.reciprocal(recip_all[:, mt:mt + 1], rsum[:])
                        kraw = kpool.tile([128, SENT], F32, tag="kraw")
                        nc.gpsimd.dma_start(
                            out=kraw[:], in_=know[mt * 128:(mt + 1) * 128, :]
                        )
                        nc.vector.tensor_scalar_mul(
                            kp[:, mt * SENT:(mt + 1) * SENT], kraw[:],
                            recip_all[:, mt:mt + 1],
                        )

                    # recip -> DRAM row, for free-axis broadcast in B2
                    rd_ap = recip_dram.opt()
                    rd_write = bass.AP(
                        tensor=rd_ap.tensor, offset=rd_ap.offset,
                        ap=[[1, 128], [128, MT]],
                    )
                    nc.sync.dma_start(out=rd_write, in_=recip_all[:])

                # ============ B1 (attention) + B2 (weights out), interleaved ======
                with ExitStack() as st3:
                    epool = st3.enter_context(tc.tile_pool(name="eb", bufs=3))
                    b2p = st3.enter_context(tc.tile_pool(name="b2p", bufs=2))
                    b2c = st3.enter_context(tc.tile_pool(name="b2c", bufs=1))
                    acc = st3.enter_context(tc.tile_pool(name="acc", bufs=1, space="PSUM"))
                    psB = st3.enter_context(tc.tile_pool(name="psB", bufs=3, space="PSUM"))

                    rbf = b2c.tile([128, M], F32)
                    rb = b2c.tile([128, M], BF16)
                    rd_ap2 = recip_dram.opt()
                    rd_bcast = bass.AP(
                        tensor=rd_ap2.tensor, offset=rd_ap2.offset,
                        ap=[[0, 128], [1, M]],
                    )
                    nc.sync.dma_start(out=rbf[:], in_=rd_bcast)
                    nc.vector.tensor_copy(rb[:], rbf[:])

                    e_ap = E_dram.opt()
                    for nch in range(NCH):
                        for half in range(2):
                            accs = [
                                acc.tile([128, 512], F32, tag=f"acc{d}",
                                         name=f"acc{d}_{half}_{nch}")
                                for d in range(4)
                            ]
                            for mtg in range(MT // 4):
                                et = epool.tile([128, 4 * 512], BF16, tag="eb")
                                src = bass.AP(
                                    tensor=e_ap.tensor,
                                    offset=e_ap.offset + (mtg * 4 * 128) * N + nch * 512,
                                    ap=[[N, 128], [128 * N, 4], [1, 512]],
                                )
                                nc.gpsimd.dma_start(out=et[:], in_=src)
                                for g in range(4):
                                    mt = mtg * 4 + g
                                    for dd in range(4):
                                        d = half * 4 + dd
                                        nc.tensor.matmul(
                                            accs[dd][:],
                                            kp[:, mt * SENT + d * 128:
                                               mt * SENT + (d + 1) * 128],
                                            et[:, g * 512:(g + 1) * 512],
                                            start=(mt == 0), stop=(mt == MT - 1),
                                        )
                            for dd in range(4):
                                d = half * 4 + dd
                                dst = attn_sb[:, d * N + nch * 512:
                                              d * N + (nch + 1) * 512]
                                if dd % 2 == 0:
                                    nc.scalar.copy(dst, accs[dd][:])
                                else:
                                    nc.vector.tensor_copy(dst, accs[dd][:])

                        # B2 slice: 4 weight row-tiles per n-chunk
                        for nt in range(nch * 4, (nch + 1) * 4):
                            for mch in range(MCH):
                                ps = psB.tile([128, 512], F32, tag="psb")
                                nc.tensor.matmul(
                                    ps[:], S_sb[:, nt * 128:(nt + 1) * 128],
                                    K_sb[:, mch * 512:(mch + 1) * 512],
                                    start=True, stop=True,
                                )
                                ex = b2p.tile([128, 512], BF16, tag="ex")
                                nc.scalar.activation(ex[:], ps[:], EXP, scale=SCALE)
                                wt = b2p.tile([128, 512], F32, tag="wt")
                                nc.vector.tensor_mul(
                                    wt[:], ex[:], rb[:, mch * 512:(mch + 1) * 512]
                                )
                                nc.sync.dma_start(
                                    out=w_out[nt * 128:(nt + 1) * 128,
                                              mch * 512:(mch + 1) * 512],
                                    in_=wt[:],
                                )

            # ============ Phase C: final projection + chunked ReduceScatter =======
            with ExitStack() as st4:
                cpool = st4.enter_context(tc.tile_pool(name="cpool", bufs=1))
                fpool = st4.enter_context(tc.tile_pool(name="fp", bufs=4))
                psC = st4.enter_context(tc.tile_pool(name="psC", bufs=6, space="PSUM"))

                Wf_sb = cpool.tile([128, DT * SENT], BF16)
                bfb8 = cpool.tile([128, SENT], F32)

                bf_ap = bf[:]
                bf_bcast = bass.AP(tensor=bf_ap.tensor, offset=bf_ap.offset,
                                   ap=[[0, 128], [1, SENT]])
                bfb_raw = fpool.tile([128, SENT], F32, tag="bfr")
                nc.sync.dma_start(out=bfb_raw[:], in_=bf_bcast)
                nc.scalar.activation(bfb8[:], bfb_raw[:], COPY, scale=1.0 / NCORES)

                for j in range(DT):
                    wfr = fpool.tile([128, SENT], F32, tag="wfr")
                    nc.sync.dma_start(out=wfr[:], in_=Wf[j * 128:(j + 1) * 128, :])
                    nc.vector.tensor_copy(Wf_sb[:, j * SENT:(j + 1) * SENT], wfr[:])

                CW = SENT // CC  # columns per RS chunk
                for cch in range(CC):
                    for nt in range(NT):
                        ps = psC.tile([128, 512], F32, tag="psc")
                        for j in range(DT):
                            nc.tensor.matmul(
                                ps[:],
                                attn_sb[:, j * N + nt * 128: j * N + nt * 128 + 128],
                                Wf_sb[:, j * SENT + cch * CW: j * SENT + (cch + 1) * CW],
                                start=(j == 0), stop=(j == DT - 1),
                            )
                        ft = fpool.tile([128, CW], BF16, tag="ft")
                        nc.vector.tensor_add(
                            ft[:], ps[:], bfb8[:, cch * CW:(cch + 1) * CW]
                        )
                        nc.sync.dma_start(
                            out=F_chunks[cch][nt * 128:(nt + 1) * 128, :], in_=ft[:]
                        )
                    nc.gpsimd.collective_compute(
                        "ReduceScatter",
                        mybir.AluOpType.add,
                        replica_groups=[list(range(NCORES))],
                        ins=[F_chunks[cch].opt()],
                        outs=[rs_chunks[cch].opt()],
                    )

                # upcast RS chunks bf16 -> fp32 and write the output rows
                for cch in range(CC):
                    for r in range(NPART // 128):
                        rt = fpool.tile([128, CW], BF16, tag="rt")
                        nc.sync.dma_start(
                            out=rt[:],
                            in_=rs_chunks[cch][r * 128:(r + 1) * 128, :],
                        )
                        ro = fpool.tile([128, CW], F32, tag="ro")
                        nc.vector.tensor_copy(ro[:], rt[:])
                        nc.sync.dma_start(
                            out=out_part[r * 128:(r + 1) * 128,
                                         cch * CW:(cch + 1) * CW],
                            in_=ro[:],
                        )

    nc.finalize()
    return nc


def kernel(sentences, knowledge, W_s, b_s, W_k, b_k, W_f, b_f):
    from concourse.bass_utils import run_bass_kernel_spmd

    if "nc" not in _CACHE:
        _CACHE["nc"] = _build()
    nc = _CACHE["nc"]

    sentences = np.ascontiguousarray(np.asarray(sentences, dtype=np.float32))
    knowledge = np.ascontiguousarray(np.asarray(knowledge, dtype=np.float32))
    sentT = np.ascontiguousarray(sentences.T)
    knowT = np.ascontiguousarray(knowledge.T)
    W_s = np.asarray(W_s, dtype=np.float32)
    b_s = np.asarray(b_s, dtype=np.float32)
    W_k = np.asarray(W_k, dtype=np.float32)
    b_k = np.asarray(b_k, dtype=np.float32)
    W_f = np.asarray(W_f, dtype=np.float32)
    b_f = np.asarray(b_f, dtype=np.float32)

    in_maps = []
    for h in range(NCORES):
        in_maps.append({
            "sentT": sentT,
            "knowT": knowT,
            "know": knowledge,
            "Ws": np.ascontiguousarray(W_s[h]),
            "bs": np.ascontiguousarray(b_s[h].reshape(HD, 1)),
            "Wk": np.ascontiguousarray(W_k[h]),
            "bk": np.ascontiguousarray(b_k[h].reshape(HD, 1)),
            "Wf": np.ascontiguousarray(W_f[h * SENT:(h + 1) * SENT, :]),
            "bf": np.ascontiguousarray(b_f.reshape(1, SENT)),
        })

    res = run_bass_kernel_spmd(nc, in_maps, list(range(NCORES)))
    output = np.concatenate(
        [res.results[i]["out_part"] for i in range(NCORES)], axis=0
    )
    weights_cat = np.concatenate(
        [res.results[i]["weights_out"] for i in range(NCORES)], axis=0
    )
    return output, weights_cat


# revision 10
# speedup vs baseline: 1.0167x; 1.0167x over previous
"""Bass/Trainium2 kernel for nn_Attention (8 heads, N=M=4096, SENT=1024, HD=128).

Strategy: head-sharded over 8 NeuronCores (core h owns head h).
Per core:
  P : S^T = (W_s^T @ sentences^T) + b_s   [128,4096] f32r   (e on partitions)
      K^T = (W_k^T @ knowledge^T) + b_k   [128,4096] f32r
  A : scoresT tiles [m,n] = K^T(slice).T @ S^T, exp(scale*x) -> E bf16 -> DRAM,
      rowsum over free axis (softmax over sentences n = free axis) via DVE on
      the bf16 E tiles, recip, K' = knowledge * recip[m]
  B1+B2 interleaved:
      B1: attn_outT[d,n] = sum_m K'[m,d] * E[m,n]  (4 PSUM banks, 2 d-halves)
      B2: weights_cat recomputed in [n,m] orientation:
          exp(S^T(slice).T @ K^T) * recip_broadcast -> fp32 -> DRAM
  C : F = attn_outT.T @ W_f_h + b_f/8 (bf16) ; column-chunked ReduceScatter
      overlapped with C; fp32 upcast of the RS result on-chip.
Host: concatenates per-core weights blocks (head-major) and RS row chunks.
"""

import numpy as np

H, SENT, HD = 8, 1024, 128
N, M = 4096, 4096
NCORES = 8
SCALE = 1.0 / float(np.sqrt(HD))

_CACHE = {}


def _build():
    from contextlib import ExitStack
    import concourse.bass as bass
    import concourse.mybir as mybir
    import concourse.tile as tile
    from concourse import bacc

    F32 = mybir.dt.float32
    F32R = mybir.dt.float32r
    BF16 = mybir.dt.bfloat16
    AX = mybir.AxisListType.X
    EXP = mybir.ActivationFunctionType.Exp
    COPY = mybir.ActivationFunctionType.Copy

    nc = bacc.Bacc("TRN2", target_bir_lowering=False, debug=False, num_devices=NCORES)

    sentT = nc.dram_tensor("sentT", [SENT, N], F32, kind="ExternalInput")
    knowT = nc.dram_tensor("knowT", [SENT, M], F32, kind="ExternalInput")
    know = nc.dram_tensor("know", [M, SENT], F32, kind="ExternalInput")
    Ws = nc.dram_tensor("Ws", [SENT, HD], F32, kind="ExternalInput")
    bs = nc.dram_tensor("bs", [HD, 1], F32, kind="ExternalInput")
    Wk = nc.dram_tensor("Wk", [SENT, HD], F32, kind="ExternalInput")
    bk = nc.dram_tensor("bk", [HD, 1], F32, kind="ExternalInput")
    Wf = nc.dram_tensor("Wf", [SENT, SENT], F32, kind="ExternalInput")
    bf = nc.dram_tensor("bf", [1, SENT], F32, kind="ExternalInput")

    w_out = nc.dram_tensor("weights_out", [N, M], F32, kind="ExternalOutput")
    out_part = nc.dram_tensor("out_part", [N // NCORES, SENT], F32, kind="ExternalOutput")

    MT = M // 128   # 32 m tiles
    NT = N // 128   # 32 n tiles
    NCH = N // 512  # 8 n chunks
    MCH = M // 512  # 8 m chunks
    DT = SENT // 128  # 8 d tiles
    NPART = N // NCORES  # 512 rows per core after RS
    CC = 2               # column chunks for overlapped ReduceScatter

    with tile.TileContext(nc) as tc:
        with ExitStack() as glob_ctx:
            glob = glob_ctx.enter_context(tc.tile_pool(name="glob", bufs=1))
            dram = glob_ctx.enter_context(tc.tile_pool(name="dram", bufs=1, space="DRAM"))

            E_dram = dram.tile([M, N], BF16)
            recip_dram = dram.tile([1, M], F32)
            F_chunks = [
                dram.tile([N, SENT // CC], BF16, name=f"Fch{c}") for c in range(CC)
            ]
            rs_chunks = [
                dram.tile([NPART, SENT // CC], BF16, name=f"rsch{c}") for c in range(CC)
            ]

            recip_all = glob.tile([128, MT], F32)
            kp = glob.tile([128, MT * SENT], BF16)   # K' blocks: [:, mt*SENT + d]
            attn_sb = glob.tile([128, DT * N], BF16)  # [:, d*N + n]
            cpool = glob_ctx.enter_context(tc.tile_pool(name="cpool", bufs=1))
            ldp = glob_ctx.enter_context(tc.tile_pool(name="ldp", bufs=1))
            Wf_sb = cpool.tile([128, DT * SENT], BF16)
            bfb8 = cpool.tile([128, SENT], F32)

            with ExitStack() as stp:
                projc = stp.enter_context(tc.tile_pool(name="projc", bufs=1))
                S_sb = projc.tile([128, N], BF16)
                K_sb = projc.tile([128, M], BF16)

                # ============ Phase P: projections (j-outer, 8 psum banks) ========
                with ExitStack() as stp2:
                    wpool = stp2.enter_context(tc.tile_pool(name="wpool", bufs=1))
                    ppool = stp2.enter_context(tc.tile_pool(name="ppool", bufs=2))
                    psP = stp2.enter_context(tc.tile_pool(name="psP", bufs=1, space="PSUM"))

                    Ws_sb = wpool.tile([128, DT * HD], BF16)
                    Wk_sb = wpool.tile([128, DT * HD], BF16)
                    bs_sb = wpool.tile([128, 1], F32)
                    bk_sb = wpool.tile([128, 1], F32)
                    nc.sync.dma_start(out=bs_sb[:], in_=bs[:])
                    nc.sync.dma_start(out=bk_sb[:], in_=bk[:])
                    for j in range(DT):
                        wr = ppool.tile([128, HD], F32, tag="wraw")
                        nc.sync.dma_start(out=wr[:], in_=Ws[j * 128:(j + 1) * 128, :])
                        nc.vector.tensor_copy(Ws_sb[:, j * HD:(j + 1) * HD], wr[:])
                        wr2 = ppool.tile([128, HD], F32, tag="wraw")
                        nc.sync.dma_start(out=wr2[:], in_=Wk[j * 128:(j + 1) * 128, :])
                        nc.vector.tensor_copy(Wk_sb[:, j * HD:(j + 1) * HD], wr2[:])

                    for tg, dst, src_dram, wsb, bcol, ldeng in (
                        ("s", S_sb, sentT, Ws_sb, bs_sb, nc.gpsimd),
                        ("k", K_sb, knowT, Wk_sb, bk_sb, nc.sync),
                    ):
                        pss = [
                            psP.tile([128, 1024], F32, tag=f"pp{q}", name=f"pp{q}")
                            for q in range(4)
                        ]
                        for j in range(DT):
                            for hh in range(4):  # 1024-col slabs
                                raw = ppool.tile([128, 1024], F32,
                                                 tag=f"praw_{tg}", name=f"praw_{tg}")
                                ldeng.dma_start(
                                    out=raw[:],
                                    in_=src_dram[j * 128:(j + 1) * 128,
                                                 hh * 1024:(hh + 1) * 1024],
                                )
                                cvt = ppool.tile([128, 1024], BF16,
                                                 tag=f"pcvt_{tg}", name=f"pcvt_{tg}")
                                nc.vector.tensor_copy(cvt[:], raw[:])
                                for qq in range(2):
                                    nc.tensor.matmul(
                                        pss[hh][:, qq * 512:(qq + 1) * 512],
                                        wsb[:, j * HD:(j + 1) * HD],
                                        cvt[:, qq * 512:(qq + 1) * 512],
                                        start=(j == 0), stop=(j == DT - 1),
                                    )
                        for q in range(4):
                            nc.vector.tensor_scalar_add(
                                dst[:, q * 1024:(q + 1) * 1024], pss[q][:], bcol[:]
                            )

                # ============ Phase A: scores/exp/rowsum + K' prep ================
                with ExitStack() as sta:
                    apool = sta.enter_context(tc.tile_pool(name="apool", bufs=2))
                    kpool = sta.enter_context(tc.tile_pool(name="kpool", bufs=3))
                    spool = sta.enter_context(tc.tile_pool(name="spool", bufs=4))
                    psA = sta.enter_context(tc.tile_pool(name="psA", bufs=4, space="PSUM"))

                    for mt in range(MT):
                        erow = apool.tile([128, N], BF16, tag="erow")
                        for nh in range(4):
                            ps = psA.tile([128, 1024], F32, tag="ps")
                            for qq in range(2):
                                nc.tensor.matmul(
                                    ps[:, qq * 512:(qq + 1) * 512],
                                    K_sb[:, mt * 128:(mt + 1) * 128],
                                    S_sb[:, nh * 1024 + qq * 512:
                                         nh * 1024 + (qq + 1) * 512],
                                    start=True, stop=True,
                                )
                            nc.scalar.activation(
                                erow[:, nh * 1024:(nh + 1) * 1024], ps[:], EXP,
                                scale=SCALE,
                            )
                        nc.sync.dma_start(
                            out=E_dram[mt * 128:(mt + 1) * 128, :], in_=erow[:]
                        )
                        rsum = spool.tile([128, 1], F32, tag="rsum")
                        nc.vector.reduce_sum(rsum[:], erow[:], axis=AX)
                        nc.vector.reciprocal(recip_all[:, mt:mt + 1], rsum[:])
                        kraw = kpool.tile([128, SENT], F32, tag="kraw")
                        nc.gpsimd.dma_start(
                            out=kraw[:], in_=know[mt * 128:(mt + 1) * 128, :]
                        )
                        nc.vector.tensor_scalar_mul(
                            kp[:, mt * SENT:(mt + 1) * SENT], kraw[:],
                            recip_all[:, mt:mt + 1],
                        )

                    # recip -> DRAM row, for free-axis broadcast in B2
                    rd_ap = recip_dram.opt()
                    rd_write = bass.AP(
                        tensor=rd_ap.tensor, offset=rd_ap.offset,
                        ap=[[1, 128], [128, MT]],
                    )
                    nc.sync.dma_start(out=rd_write, in_=recip_all[:])

                # ============ B1 (attention) + B2 (weights out), interleaved ======
                with ExitStack() as st3:
                    epool = st3.enter_context(tc.tile_pool(name="eb", bufs=3))
                    b2p = st3.enter_context(tc.tile_pool(name="b2p", bufs=2))
                    b2c = st3.enter_context(tc.tile_pool(name="b2c", bufs=1))
                    acc = st3.enter_context(tc.tile_pool(name="acc", bufs=1, space="PSUM"))
                    psB = st3.enter_context(tc.tile_pool(name="psB", bufs=3, space="PSUM"))

                    rb = b2c.tile([128, M], BF16)
                    rd_ap2 = recip_dram.opt()
                    for rc in range(4):
                        rbc = ldp.tile([128, 1024], F32, tag="rbc")
                        rd_bcast = bass.AP(
                            tensor=rd_ap2.tensor, offset=rd_ap2.offset + rc * 1024,
                            ap=[[0, 128], [1, 1024]],
                        )
                        nc.sync.dma_start(out=rbc[:], in_=rd_bcast)
                        nc.vector.tensor_copy(rb[:, rc * 1024:(rc + 1) * 1024], rbc[:])

                    e_ap = E_dram.opt()
                    for nch in range(NCH):
                        if nch == NCH - 1:
                            bf_ap = bf[:]
                            bf_bcast = bass.AP(
                                tensor=bf_ap.tensor, offset=bf_ap.offset,
                                ap=[[0, 128], [1, SENT]],
                            )
                            bfb_raw = ldp.tile([128, SENT], F32, tag="bfr")
                            nc.sync.dma_start(out=bfb_raw[:], in_=bf_bcast)
                            nc.scalar.activation(bfb8[:], bfb_raw[:], COPY,
                                                 scale=1.0 / NCORES)
                            for j in range(DT):
                                wfr = ldp.tile([128, SENT], F32, tag="wfr")
                                nc.sync.dma_start(
                                    out=wfr[:], in_=Wf[j * 128:(j + 1) * 128, :]
                                )
                                nc.vector.tensor_copy(
                                    Wf_sb[:, j * SENT:(j + 1) * SENT], wfr[:]
                                )
                        for half in range(2):
                            accs = [
                                acc.tile([128, 512], F32, tag=f"acc{d}",
                                         name=f"acc{d}_{half}_{nch}")
                                for d in range(4)
                            ]
                            for mtg in range(MT // 4):
                                et = epool.tile([128, 4 * 512], BF16, tag="eb")
                                src = bass.AP(
                                    tensor=e_ap.tensor,
                                    offset=e_ap.offset + (mtg * 4 * 128) * N + nch * 512,
                                    ap=[[N, 128], [128 * N, 4], [1, 512]],
                                )
                                nc.gpsimd.dma_start(out=et[:], in_=src)
                                for g in range(4):
                                    mt = mtg * 4 + g
                                    for dd in range(4):
                                        d = half * 4 + dd
                                        nc.tensor.matmul(
                                            accs[dd][:],
                                            kp[:, mt * SENT + d * 128:
                                               mt * SENT + (d + 1) * 128],
                                            et[:, g * 512:(g + 1) * 512],
                                            start=(mt == 0), stop=(mt == MT - 1),
                                        )
                            for dd in range(4):
                                d = half * 4 + dd
                                dst = attn_sb[:, d * N + nch * 512:
                                              d * N + (nch + 1) * 512]
                                if dd % 2 == 0:
                                    nc.scalar.copy(dst, accs[dd][:])
                                else:
                                    nc.vector.tensor_copy(dst, accs[dd][:])

                        # B2 slice: 4 weight row-tiles per n-chunk
                        for nt in range(nch * 4, (nch + 1) * 4):
                            for mch in range(MCH):
                                ps = psB.tile([128, 512], F32, tag="psb")
                                nc.tensor.matmul(
                                    ps[:], S_sb[:, nt * 128:(nt + 1) * 128],
                                    K_sb[:, mch * 512:(mch + 1) * 512],
                                    start=True, stop=True,
                                )
                                ex = b2p.tile([128, 512], BF16, tag="ex")
                                nc.scalar.activation(ex[:], ps[:], EXP, scale=SCALE)
                                wt = b2p.tile([128, 512], F32, tag="wt")
                                nc.vector.tensor_mul(
                                    wt[:], ex[:], rb[:, mch * 512:(mch + 1) * 512]
                                )
                                nc.sync.dma_start(
                                    out=w_out[nt * 128:(nt + 1) * 128,
                                              mch * 512:(mch + 1) * 512],
                                    in_=wt[:],
                                )

            # ============ Phase C: final projection + chunked ReduceScatter =======
            with ExitStack() as st4:
                fpool = st4.enter_context(tc.tile_pool(name="fp", bufs=4))
                psC = st4.enter_context(tc.tile_pool(name="psC", bufs=6, space="PSUM"))

                CW = SENT // CC  # columns per RS chunk
                for cch in range(CC):
                    for nt in range(NT):
                        ps = psC.tile([128, 512], F32, tag="psc")
                        for j in range(DT):
                            nc.tensor.matmul(
                                ps[:],
                                attn_sb[:, j * N + nt * 128: j * N + nt * 128 + 128],
                                Wf_sb[:, j * SENT + cch * CW: j * SENT + (cch + 1) * CW],
                                start=(j == 0), stop=(j == DT - 1),
                            )
                        ft = fpool.tile([128, CW], BF16, tag="ft")
                        nc.vector.tensor_add(
                            ft[:], ps[:], bfb8[:, cch * CW:(cch + 1) * CW]
                        )
                        nc.sync.dma_start(
                            out=F_chunks[cch][nt * 128:(nt + 1) * 128, :], in_=ft[:]
                        )
                    nc.gpsimd.collective_compute(
                        "ReduceScatter",
                        mybir.AluOpType.add,
                        replica_groups=[list(range(NCORES))],
                        ins=[F_chunks[cch].opt()],
                        outs=[rs_chunks[cch].opt()],
                    )
                    # upcast this chunk right away (overlaps next chunk compute)
                    for r in range(NPART // 128):
                        rt = fpool.tile([128, CW], BF16, tag="rt")
                        nc.sync.dma_start(
                            out=rt[:],
                            in_=rs_chunks[cch][r * 128:(r + 1) * 128, :],
                        )
                        ro = fpool.tile([128, CW], F32, tag="ro")
                        nc.vector.tensor_copy(ro[:], rt[:])
                        nc.sync.dma_start(
                            out=out_part[r * 128:(r + 1) * 128,
                                         cch * CW:(cch + 1) * CW],
                            in_=ro[:],
                        )

    nc.finalize()
    return nc


def kernel(sentences, knowledge, W_s, b_s, W_k, b_k, W_f, b_f):
    from concourse.bass_utils import run_bass_kernel_spmd

    if "nc" not in _CACHE:
        _CACHE["nc"] = _build()
    nc = _CACHE["nc"]

    sentences = np.ascontiguousarray(np.asarray(sentences, dtype=np.float32))
    knowledge = np.ascontiguousarray(np.asarray(knowledge, dtype=np.float32))
    sentT = np.ascontiguousarray(sentences.T)
    knowT = np.ascontiguousarray(knowledge.T)
    W_s = np.asarray(W_s, dtype=np.float32)
    b_s = np.asarray(b_s, dtype=np.float32)
    W_k = np.asarray(W_k, dtype=np.float32)
    b_k = np.asarray(b_k, dtype=np.float32)
    W_f = np.asarray(W_f, dtype=np.float32)
    b_f = np.asarray(b_f, dtype=np.float32)

    in_maps = []
    for h in range(NCORES):
        in_maps.append({
            "sentT": sentT,
            "knowT": knowT,
            "know": knowledge,
            "Ws": np.ascontiguousarray(W_s[h]),
            "bs": np.ascontiguousarray(b_s[h].reshape(HD, 1)),
            "Wk": np.ascontiguousarray(W_k[h]),
            "bk": np.ascontiguousarray(b_k[h].reshape(HD, 1)),
            "Wf": np.ascontiguousarray(W_f[h * SENT:(h + 1) * SENT, :]),
            "bf": np.ascontiguousarray(b_f.reshape(1, SENT)),
        })

    res = run_bass_kernel_spmd(nc, in_maps, list(range(NCORES)))
    output = np.concatenate(
        [res.results[i]["out_part"] for i in range(NCORES)], axis=0
    )
    weights_cat = np.concatenate(
        [res.results[i]["weights_out"] for i in range(NCORES)], axis=0
    )
    return output, weights_cat


# revision 12
# speedup vs baseline: 1.1400x; 1.1214x over previous
"""Bass/Trainium2 kernel for nn_Attention (8 heads, N=M=4096, SENT=1024, HD=128).

Strategy: head-sharded over 8 NeuronCores (core h owns head h).
Per core:
  P : S^T = (W_s^T @ sentences^T) + b_s   [128,4096] f32r   (e on partitions)
      K^T = (W_k^T @ knowledge^T) + b_k   [128,4096] f32r
  A : scoresT tiles [m,n] = K^T(slice).T @ S^T, exp(scale*x) -> E bf16 -> DRAM,
      rowsum over free axis (softmax over sentences n = free axis) via DVE on
      the bf16 E tiles, recip, K' = knowledge * recip[m]
  B1+B2 interleaved:
      B1: attn_outT[d,n] = sum_m K'[m,d] * E[m,n]  (4 PSUM banks, 2 d-halves)
      B2: weights_cat recomputed in [n,m] orientation:
          exp(S^T(slice).T @ K^T) * recip_broadcast -> fp32 -> DRAM
  C : F = attn_outT.T @ W_f_h + b_f/8 (bf16) ; column-chunked ReduceScatter
      overlapped with C; fp32 upcast of the RS result on-chip.
Host: concatenates per-core weights blocks (head-major) and RS row chunks.
"""

import numpy as np

H, SENT, HD = 8, 1024, 128
N, M = 4096, 4096
NCORES = 8
SCALE = 1.0 / float(np.sqrt(HD))

_CACHE = {}


def _build():
    from contextlib import ExitStack
    import concourse.bass as bass
    import concourse.mybir as mybir
    import concourse.tile as tile
    from concourse import bacc

    F32 = mybir.dt.float32
    F32R = mybir.dt.float32r
    BF16 = mybir.dt.bfloat16
    AX = mybir.AxisListType.X
    EXP = mybir.ActivationFunctionType.Exp
    COPY = mybir.ActivationFunctionType.Copy

    nc = bacc.Bacc("TRN2", target_bir_lowering=False, debug=False, num_devices=NCORES)

    sentT = nc.dram_tensor("sentT", [SENT, N], BF16, kind="ExternalInput")
    knowT = nc.dram_tensor("knowT", [SENT, M], BF16, kind="ExternalInput")
    know = nc.dram_tensor("know", [M, SENT], BF16, kind="ExternalInput")
    Ws = nc.dram_tensor("Ws", [SENT, HD], F32, kind="ExternalInput")
    bs = nc.dram_tensor("bs", [HD, 1], F32, kind="ExternalInput")
    Wk = nc.dram_tensor("Wk", [SENT, HD], F32, kind="ExternalInput")
    bk = nc.dram_tensor("bk", [HD, 1], F32, kind="ExternalInput")
    Wf = nc.dram_tensor("Wf", [SENT, SENT], BF16, kind="ExternalInput")
    bf = nc.dram_tensor("bf", [1, SENT], F32, kind="ExternalInput")

    w_out = nc.dram_tensor("weights_out", [N, M], F32, kind="ExternalOutput")
    out_part = nc.dram_tensor("out_part", [N // NCORES, SENT], F32, kind="ExternalOutput")

    MT = M // 128   # 32 m tiles
    NT = N // 128   # 32 n tiles
    NCH = N // 512  # 8 n chunks
    MCH = M // 512  # 8 m chunks
    DT = SENT // 128  # 8 d tiles
    NPART = N // NCORES  # 512 rows per core after RS
    CC = 2               # column chunks for overlapped ReduceScatter

    with tile.TileContext(nc) as tc:
        with ExitStack() as glob_ctx:
            glob = glob_ctx.enter_context(tc.tile_pool(name="glob", bufs=1))
            dram = glob_ctx.enter_context(tc.tile_pool(name="dram", bufs=1, space="DRAM"))

            E_dram = dram.tile([M, N], BF16)
            recip_dram = dram.tile([1, M], F32)
            F_chunks = [
                dram.tile([N, SENT // CC], BF16, name=f"Fch{c}") for c in range(CC)
            ]
            rs_chunks = [
                dram.tile([NPART, SENT // CC], BF16, name=f"rsch{c}") for c in range(CC)
            ]

            recip_all = glob.tile([128, MT], F32)
            kp = glob.tile([128, MT * SENT], BF16)   # K' blocks: [:, mt*SENT + d]
            attn_sb = glob.tile([128, DT * N], BF16)  # [:, d*N + n]
            cpool = glob_ctx.enter_context(tc.tile_pool(name="cpool", bufs=1))
            ldp = glob_ctx.enter_context(tc.tile_pool(name="ldp", bufs=1))
            Wf_sb = cpool.tile([128, DT * SENT], BF16)
            bfb8 = cpool.tile([128, SENT], F32)

            with ExitStack() as stp:
                projc = stp.enter_context(tc.tile_pool(name="projc", bufs=1))
                S_sb = projc.tile([128, N], BF16)
                K_sb = projc.tile([128, M], BF16)

                # ============ Phase P: projections (j-outer, 8 psum banks) ========
                with ExitStack() as stp2:
                    wpool = stp2.enter_context(tc.tile_pool(name="wpool", bufs=1))
                    ppool = stp2.enter_context(tc.tile_pool(name="ppool", bufs=4))
                    psP = stp2.enter_context(tc.tile_pool(name="psP", bufs=1, space="PSUM"))

                    Ws_sb = wpool.tile([128, DT * HD], BF16)
                    Wk_sb = wpool.tile([128, DT * HD], BF16)
                    bs_sb = wpool.tile([128, 1], F32)
                    bk_sb = wpool.tile([128, 1], F32)
                    nc.sync.dma_start(out=bs_sb[:], in_=bs[:])
                    nc.sync.dma_start(out=bk_sb[:], in_=bk[:])
                    for j in range(DT):
                        wr = ppool.tile([128, HD], F32, tag="wraw")
                        nc.sync.dma_start(out=wr[:], in_=Ws[j * 128:(j + 1) * 128, :])
                        nc.vector.tensor_copy(Ws_sb[:, j * HD:(j + 1) * HD], wr[:])
                        wr2 = ppool.tile([128, HD], F32, tag="wraw")
                        nc.sync.dma_start(out=wr2[:], in_=Wk[j * 128:(j + 1) * 128, :])
                        nc.vector.tensor_copy(Wk_sb[:, j * HD:(j + 1) * HD], wr2[:])

                    for tg, dst, src_dram, wsb, bcol, ldeng in (
                        ("s", S_sb, sentT, Ws_sb, bs_sb, nc.gpsimd),
                        ("k", K_sb, knowT, Wk_sb, bk_sb, nc.sync),
                    ):
                        pss = [
                            psP.tile([128, 1024], F32, tag=f"pp{q}", name=f"pp{q}")
                            for q in range(4)
                        ]
                        for j in range(DT):
                            for hh in range(4):  # 1024-col slabs
                                cvt = ppool.tile([128, 1024], BF16,
                                                 tag=f"pcvt_{tg}", name=f"pcvt_{tg}")
                                ldeng.dma_start(
                                    out=cvt[:],
                                    in_=src_dram[j * 128:(j + 1) * 128,
                                                 hh * 1024:(hh + 1) * 1024],
                                )
                                for qq in range(2):
                                    nc.tensor.matmul(
                                        pss[hh][:, qq * 512:(qq + 1) * 512],
                                        wsb[:, j * HD:(j + 1) * HD],
                                        cvt[:, qq * 512:(qq + 1) * 512],
                                        start=(j == 0), stop=(j == DT - 1),
                                    )
                        for q in range(4):
                            nc.vector.tensor_scalar_add(
                                dst[:, q * 1024:(q + 1) * 1024], pss[q][:], bcol[:]
                            )

                # ============ Phase A: scores/exp/rowsum + K' prep ================
                with ExitStack() as sta:
                    apool = sta.enter_context(tc.tile_pool(name="apool", bufs=2))
                    kpool = sta.enter_context(tc.tile_pool(name="kpool", bufs=3))
                    spool = sta.enter_context(tc.tile_pool(name="spool", bufs=4))
                    psA = sta.enter_context(tc.tile_pool(name="psA", bufs=2, space="PSUM"))

                    for mt in range(MT):
                        erow = apool.tile([128, N], BF16, tag="erow")
                        for nh in range(2):
                            ps = psA.tile([128, 2048], F32, tag="ps")
                            for qq in range(4):
                                nc.tensor.matmul(
                                    ps[:, qq * 512:(qq + 1) * 512],
                                    K_sb[:, mt * 128:(mt + 1) * 128],
                                    S_sb[:, nh * 2048 + qq * 512:
                                         nh * 2048 + (qq + 1) * 512],
                                    start=True, stop=True,
                                )
                            nc.scalar.activation(
                                erow[:, nh * 2048:(nh + 1) * 2048], ps[:], EXP,
                                scale=SCALE,
                            )
                        nc.sync.dma_start(
                            out=E_dram[mt * 128:(mt + 1) * 128, :], in_=erow[:]
                        )
                        rsum = spool.tile([128, 1], F32, tag="rsum")
                        nc.vector.reduce_sum(rsum[:], erow[:], axis=AX)
                        nc.vector.reciprocal(recip_all[:, mt:mt + 1], rsum[:])
                        kraw = kpool.tile([128, SENT], BF16, tag="kraw")
                        nc.gpsimd.dma_start(
                            out=kraw[:], in_=know[mt * 128:(mt + 1) * 128, :]
                        )
                        nc.vector.tensor_scalar_mul(
                            kp[:, mt * SENT:(mt + 1) * SENT], kraw[:],
                            recip_all[:, mt:mt + 1],
                        )

                    # recip -> DRAM row, for free-axis broadcast in B2
                    rd_ap = recip_dram.opt()
                    rd_write = bass.AP(
                        tensor=rd_ap.tensor, offset=rd_ap.offset,
                        ap=[[1, 128], [128, MT]],
                    )
                    nc.sync.dma_start(out=rd_write, in_=recip_all[:])

                # ============ B1 (attention) + B2 (weights out), interleaved ======
                with ExitStack() as st3:
                    epool = st3.enter_context(tc.tile_pool(name="eb", bufs=3))
                    b2p = st3.enter_context(tc.tile_pool(name="b2p", bufs=2))
                    b2c = st3.enter_context(tc.tile_pool(name="b2c", bufs=1))
                    acc = st3.enter_context(tc.tile_pool(name="acc", bufs=1, space="PSUM"))
                    psB = st3.enter_context(tc.tile_pool(name="psB", bufs=2, space="PSUM"))
                    psC0 = st3.enter_context(tc.tile_pool(name="psC0", bufs=2, space="PSUM"))
                    ftp = st3.enter_context(tc.tile_pool(name="ftp", bufs=3))
                    CW = SENT // CC

                    rb = b2c.tile([128, M], BF16)
                    rd_ap2 = recip_dram.opt()
                    for rc in range(4):
                        rbc = ldp.tile([128, 1024], F32, tag="rbc")
                        rd_bcast = bass.AP(
                            tensor=rd_ap2.tensor, offset=rd_ap2.offset + rc * 1024,
                            ap=[[0, 128], [1, 1024]],
                        )
                        nc.sync.dma_start(out=rbc[:], in_=rd_bcast)
                        nc.vector.tensor_copy(rb[:, rc * 1024:(rc + 1) * 1024], rbc[:])

                    bf_ap = bf[:]
                    bf_bcast = bass.AP(
                        tensor=bf_ap.tensor, offset=bf_ap.offset,
                        ap=[[0, 128], [1, SENT]],
                    )
                    bfb_raw = ldp.tile([128, SENT], F32, tag="bfr")
                    nc.sync.dma_start(out=bfb_raw[:], in_=bf_bcast)
                    nc.scalar.activation(bfb8[:], bfb_raw[:], COPY,
                                         scale=1.0 / NCORES)
                    for j in range(DT):
                        nc.sync.dma_start(
                            out=Wf_sb[:, j * SENT:(j + 1) * SENT],
                            in_=Wf[j * 128:(j + 1) * 128, :],
                        )
                    e_ap = E_dram.opt()
                    for nch in range(NCH):
                        for half in range(2):
                            accs = [
                                acc.tile([128, 512], F32, tag=f"acc{d}",
                                         name=f"acc{d}_{half}_{nch}")
                                for d in range(4)
                            ]
                            for mtg in range(MT // 4):
                                et = epool.tile([128, 4 * 512], BF16, tag="eb")
                                src = bass.AP(
                                    tensor=e_ap.tensor,
                                    offset=e_ap.offset + (mtg * 4 * 128) * N + nch * 512,
                                    ap=[[N, 128], [128 * N, 4], [1, 512]],
                                )
                                nc.gpsimd.dma_start(out=et[:], in_=src)
                                for g in range(4):
                                    mt = mtg * 4 + g
                                    for dd in range(4):
                                        d = half * 4 + dd
                                        nc.tensor.matmul(
                                            accs[dd][:],
                                            kp[:, mt * SENT + d * 128:
                                               mt * SENT + (d + 1) * 128],
                                            et[:, g * 512:(g + 1) * 512],
                                            start=(mt == 0), stop=(mt == MT - 1),
                                        )
                            for dd in range(4):
                                d = half * 4 + dd
                                dst = attn_sb[:, d * N + nch * 512:
                                              d * N + (nch + 1) * 512]
                                if dd % 2 == 0:
                                    nc.scalar.copy(dst, accs[dd][:])
                                else:
                                    nc.vector.tensor_copy(dst, accs[dd][:])

                        # B2 slice: 4 weight row-tiles per n-chunk
                        for nt in range(nch * 4, (nch + 1) * 4):
                            for mch in range(MCH):
                                ps = psB.tile([128, 512], F32, tag="psb")
                                nc.tensor.matmul(
                                    ps[:], S_sb[:, nt * 128:(nt + 1) * 128],
                                    K_sb[:, mch * 512:(mch + 1) * 512],
                                    start=True, stop=True,
                                )
                                ex = b2p.tile([128, 512], BF16, tag="ex")
                                nc.scalar.activation(ex[:], ps[:], EXP, scale=SCALE)
                                wt = b2p.tile([128, 512], F32, tag="wt")
                                nc.vector.tensor_mul(
                                    wt[:], ex[:], rb[:, mch * 512:(mch + 1) * 512]
                                )
                                nc.sync.dma_start(
                                    out=w_out[nt * 128:(nt + 1) * 128,
                                              mch * 512:(mch + 1) * 512],
                                    in_=wt[:],
                                )

                        # C chunk 0 rows for this n-chunk (fills PE gaps)
                        for nt in range(nch * 4, (nch + 1) * 4):
                            psf = psC0.tile([128, CW], F32, tag="psc0")
                            for j in range(DT):
                                nc.tensor.matmul(
                                    psf[:],
                                    attn_sb[:, j * N + nt * 128: j * N + nt * 128 + 128],
                                    Wf_sb[:, j * SENT: j * SENT + CW],
                                    start=(j == 0), stop=(j == DT - 1),
                                )
                            ft0 = ftp.tile([128, CW], BF16, tag="ft0")
                            nc.vector.tensor_add(ft0[:], psf[:], bfb8[:, 0:CW])
                            nc.sync.dma_start(
                                out=F_chunks[0][nt * 128:(nt + 1) * 128, :], in_=ft0[:]
                            )

                    nc.gpsimd.collective_compute(
                        "ReduceScatter",
                        mybir.AluOpType.add,
                        replica_groups=[list(range(NCORES))],
                        ins=[F_chunks[0].opt()],
                        outs=[rs_chunks[0].opt()],
                    )

            # ============ Phase C: final projection + chunked ReduceScatter =======
            with ExitStack() as st4:
                fpool = st4.enter_context(tc.tile_pool(name="fp", bufs=4))
                psC = st4.enter_context(tc.tile_pool(name="psC", bufs=4, space="PSUM"))

                CW1 = SENT // CC
                for nt in range(NT):
                    ps = psC.tile([128, CW1], F32, tag="psc")
                    for j in range(DT):
                        nc.tensor.matmul(
                            ps[:],
                            attn_sb[:, j * N + nt * 128: j * N + nt * 128 + 128],
                            Wf_sb[:, j * SENT + CW1: j * SENT + 2 * CW1],
                            start=(j == 0), stop=(j == DT - 1),
                        )
                    ft = fpool.tile([128, CW1], BF16, tag="ft")
                    nc.vector.tensor_add(ft[:], ps[:], bfb8[:, CW1:2 * CW1])
                    nc.sync.dma_start(
                        out=F_chunks[1][nt * 128:(nt + 1) * 128, :], in_=ft[:]
                    )
                nc.gpsimd.collective_compute(
                    "ReduceScatter",
                    mybir.AluOpType.add,
                    replica_groups=[list(range(NCORES))],
                    ins=[F_chunks[1].opt()],
                    outs=[rs_chunks[1].opt()],
                )

                # upcast RS chunks bf16 -> fp32 and write the output rows
                for cch in range(CC):
                    for r in range(NPART // 128):
                        rt = fpool.tile([128, CW1], BF16, tag="rt")
                        nc.sync.dma_start(
                            out=rt[:],
                            in_=rs_chunks[cch][r * 128:(r + 1) * 128, :],
                        )
                        ro = fpool.tile([128, CW1], F32, tag="ro")
                        nc.vector.tensor_copy(ro[:], rt[:])
                        nc.sync.dma_start(
                            out=out_part[r * 128:(r + 1) * 128,
                                         cch * CW1:(cch + 1) * CW1],
                            in_=ro[:],
                        )

    nc.finalize()
    return nc


def kernel(sentences, knowledge, W_s, b_s, W_k, b_k, W_f, b_f):
    from concourse.bass_utils import run_bass_kernel_spmd

    if "nc" not in _CACHE:
        _CACHE["nc"] = _build()
    nc = _CACHE["nc"]

    import ml_dtypes
    bfdt = ml_dtypes.bfloat16
    sentences = np.ascontiguousarray(np.asarray(sentences, dtype=np.float32))
    knowledge = np.ascontiguousarray(np.asarray(knowledge, dtype=np.float32))
    sentT = np.ascontiguousarray(sentences.T.astype(bfdt))
    knowT = np.ascontiguousarray(knowledge.T.astype(bfdt))
    know_bf = np.ascontiguousarray(knowledge.astype(bfdt))
    W_s = np.asarray(W_s, dtype=np.float32)
    b_s = np.asarray(b_s, dtype=np.float32)
    W_k = np.asarray(W_k, dtype=np.float32)
    b_k = np.asarray(b_k, dtype=np.float32)
    W_f = np.asarray(W_f, dtype=np.float32)
    b_f = np.asarray(b_f, dtype=np.float32)

    in_maps = []
    for h in range(NCORES):
        in_maps.append({
            "sentT": sentT,
            "knowT": knowT,
            "know": know_bf,
            "Ws": np.ascontiguousarray(W_s[h]),
            "bs": np.ascontiguousarray(b_s[h].reshape(HD, 1)),
            "Wk": np.ascontiguousarray(W_k[h]),
            "bk": np.ascontiguousarray(b_k[h].reshape(HD, 1)),
            "Wf": np.ascontiguousarray(W_f[h * SENT:(h + 1) * SENT, :].astype(bfdt)),
            "bf": np.ascontiguousarray(b_f.reshape(1, SENT)),
        })

    res = run_bass_kernel_spmd(nc, in_maps, list(range(NCORES)))
    output = np.concatenate(
        [res.results[i]["out_part"] for i in range(NCORES)], axis=0
    )
    weights_cat = np.concatenate(
        [res.results[i]["weights_out"] for i in range(NCORES)], axis=0
    )
    return output, weights_cat
